# revision 1
# baseline (speedup 1.0000x reference)
"""Multi-head self-attention (B=4, N=2048, D=768, H=12, dh=64) on 8 Trainium2
NeuronCores.

Sharding: core c handles batch b = c // 2 and heads [6*(c%2), 6*(c%2)+6).
Each core computes its 6 heads' Q/K/V projections, attention, and a partial
output projection (its 384 rows of w_o). The host sums the two partials per
batch element and adds b_o.

Per-core kernel (all matmuls bf16 inputs, fp32 PSUM accumulate):
  xT   [768, 2048]  via PE transpose (fp32 in, bf16 out through PSUM copy)
  QT/KT [384, 2048] = w.T @ x.T       (heads on partitions, 64 rows each)
  V    [2048, 390]  = x @ w_v         (+ ones column per head for softmax denom)
  per head, per 512-query chunk, per kv-tile group:
    S^T  [kv=128, q=512] = K Q^T      (PSUM)
    P^T  = exp(S^T / 8)               (ScalarE, bf16 out, denom-safe range)
    O'^T [65, 512] += V'^T P^T        (PSUM accumulate; row 64 = softmax denom)
  H^T  = O'^T[0:64] * recip(denom)    (DVE + gpsimd partition_broadcast)
  out  [2048, 768] = H^T.T @ w_o_part (fp32 out)
"""
import sys

if "/opt/trn_rl_repo" not in sys.path:
    sys.path.insert(0, "/opt/trn_rl_repo")

import numpy as np

import concourse.bass as bass
import concourse.tile as tile
from concourse import bacc, mybir
from concourse.masks import make_identity

P = 128
B, N, D = 4, 2048, 768
HEADS, DH = 12, 64
HL = 6                 # heads per core
INNER_L = HL * DH      # 384 local inner dim
DC = D // P            # 6 chunks of model dim
IC = INNER_L // P      # 3 chunks of local inner dim
NT = N // P            # 16 token tiles
NQ = 512               # query chunk
QC = N // NQ           # 4 query chunks
KV_GROUPS = [3, 3, 3, 3, 3, 1]   # kv-tile grouping for batched exp (sums to 16)

F32 = mybir.dt.float32
BF = mybir.dt.bfloat16

_CACHED_NC = None


def build_program(reps=1, loop_n=0, phases="ABCD"):
    nc = bacc.Bacc("TRN2", target_bir_lowering=False, debug=False)

    x_d = nc.dram_tensor("x", [N, D], F32, kind="ExternalInput").ap()
    wq_d = nc.dram_tensor("w_q", [D, INNER_L], F32, kind="ExternalInput").ap()
    wk_d = nc.dram_tensor("w_k", [D, INNER_L], F32, kind="ExternalInput").ap()
    wv_d = nc.dram_tensor("w_v", [D, INNER_L], F32, kind="ExternalInput").ap()
    wo_d = nc.dram_tensor("w_o", [INNER_L, D], F32, kind="ExternalInput").ap()
    out_d = nc.dram_tensor("out", [N, D], F32, kind="ExternalOutput").ap()

    with tile.TileContext(nc) as tc:
        if loop_n:
            with tc.For_i(0, loop_n, 1):
                _build_body(nc, tc, 0, x_d, wq_d, wk_d, wv_d, wo_d, out_d, phases)
        else:
            for rep in range(reps):
                _build_body(nc, tc, rep, x_d, wq_d, wk_d, wv_d, wo_d, out_d, phases)
    nc.compile()
    return nc


def _build_body(nc, tc, rep, x_d, wq_d, wk_d, wv_d, wo_d, out_d, phases="ABCD"):
    if True:
        with tc.tile_pool(name=f"persist{rep}", bufs=1) as persist:
            xT = persist.tile([P, DC, N], BF)        # x^T, D on partitions
            QT = persist.tile([P, IC, N], BF)        # Q^T, inner on partitions
            KT = persist.tile([P, IC, N], BF)
            V = persist.tile([P, NT, HL, DH + 1], BF)  # [t, head, 64 v + ones]
            HT = persist.tile([P, IC, N], BF)        # normalized head outputs^T
            wq_sb = persist.tile([P, DC, INNER_L], BF)
            wk_sb = persist.tile([P, DC, INNER_L], BF)
            wv_sb = persist.tile([P, DC, INNER_L], BF)
            wo_sb = persist.tile([P, IC, D], BF)
            ident = persist.tile([P, P], F32)

            make_identity(nc, ident)
            nc.vector.memset(V[:, :, :, DH:DH + 1], 1.0)

            # ---- Phase A: load + cast weights, load x, transpose to xT ----
            if "A" in phases:
              with (
                tc.tile_pool(name=f"stageA{rep}", bufs=1) as stageA,
                tc.tile_pool(name=f"wstage{rep}", bufs=2) as wstage,
                tc.tile_pool(name=f"psA{rep}", bufs=4, space="PSUM") as psA,
              ):
                x_f32 = stageA.tile([P, NT, D], F32)
                nc.sync.dma_start(x_f32, x_d.rearrange("(kt p) d -> p kt d", p=P))

                for w_d_ap, w_sb in ((wq_d, wq_sb), (wk_d, wk_sb), (wv_d, wv_sb)):
                    w_f32 = wstage.tile([P, DC, INNER_L], F32, tag="wf")
                    nc.sync.dma_start(w_f32, w_d_ap.rearrange("(c p) i -> p c i", p=P))
                    nc.vector.tensor_copy(w_sb, w_f32)
                wo_f32 = wstage.tile([P, IC, D], F32, tag="wf")
                nc.sync.dma_start(wo_f32, wo_d.rearrange("(c p) o -> p c o", p=P))
                nc.vector.tensor_copy(wo_sb, wo_f32)

                for c in range(DC):
                    for kt in range(NT):
                        tp = psA.tile([P, P], F32, tag="tp")
                        nc.tensor.transpose(
                            tp, x_f32[:, kt, c * P:(c + 1) * P], ident
                        )
                        nc.vector.tensor_copy(xT[:, c, kt * P:(kt + 1) * P], tp)

            # ---- Phase B+C: projections merged into attention scope ----
            if "C" in phases:
              with (
                tc.tile_pool(name=f"psS{rep}", bufs=1, space="PSUM") as psS,
                tc.tile_pool(name=f"psO{rep}", bufs=1, space="PSUM") as psO,
                tc.tile_pool(name=f"csb{rep}", bufs=2) as csb,
              ):
                # projections use the O-accumulator banks (tags oe/oo) before
                # attention claims them; head-pair 0's Q/K go first so exp
                # work can start while later projections still run on PE.
                ptag = ["oe", "oo"]
                pcnt = 0

                def proj_qk(mc):
                    nonlocal pcnt
                    for w_sb, dst in ((wq_sb, QT), (wk_sb, KT)):
                        for qc in range(QC):
                            pp = psO.tile([P, NQ], F32, tag=ptag[pcnt % 2],
                                          name=f"pp{mc}")
                            pcnt += 1
                            for c in range(DC):
                                nc.tensor.matmul(
                                    pp,
                                    w_sb[:, c, mc * P:(mc + 1) * P],
                                    xT[:, c, qc * NQ:(qc + 1) * NQ],
                                    start=(c == 0),
                                    stop=(c == DC - 1),
                                )
                            nc.vector.tensor_copy(
                                dst[:, mc, qc * NQ:(qc + 1) * NQ], pp
                            )

                proj_qk(0)
                for kt in range(NT):
                    pv = psO.tile([P, NQ], F32, tag=ptag[pcnt % 2], name="pv")
                    pcnt += 1
                    for c in range(DC):
                        nc.tensor.matmul(
                            pv[:, 0:INNER_L],
                            xT[:, c, kt * P:(kt + 1) * P],
                            wv_sb[:, c, :],
                            start=(c == 0),
                            stop=(c == DC - 1),
                        )
                    nc.vector.tensor_copy(
                        V[:, kt, :, 0:DH],
                        pv[:, 0:INNER_L].rearrange("p (h d) -> p h d", h=HL),
                    )
                proj_qk(1)
                proj_qk(2)

                for qc in range(QC):
                    qsl = slice(qc * NQ, (qc + 1) * NQ)
                    for hp in range(IC):  # head pair = inner chunk
                        o_e = psO.tile([P, NQ], F32, tag="oe")
                        o_o = psO.tile([P, NQ], F32, tag="oo")
                        kt0 = 0
                        for g, glen in enumerate(KV_GROUPS):
                            s_e = psS.tile([P, 3, NQ], F32, tag="se")
                            s_o = psS.tile([P, 3, NQ], F32, tag="so")
                            for j in range(glen):
                                kt = kt0 + j
                                ksl = slice(kt * P, (kt + 1) * P)
                                nc.tensor.matmul(
                                    s_e[:, j], KT[0:DH, hp, ksl],
                                    QT[0:DH, hp, qsl], start=True, stop=True,
                                )
                                nc.tensor.matmul(
                                    s_o[:, j], KT[DH:P, hp, ksl],
                                    QT[DH:P, hp, qsl], start=True, stop=True,
                                )
                            p_e = csb.tile([P, 3, NQ], BF, tag="pe", bufs=10)
                            p_o = csb.tile([P, 3, NQ], BF, tag="po", bufs=10)
                            nc.scalar.activation(
                                p_e[:, 0:glen], s_e[:, 0:glen],
                                mybir.ActivationFunctionType.Exp, scale=0.125,
                            )
                            nc.scalar.activation(
                                p_o[:, 0:glen], s_o[:, 0:glen],
                                mybir.ActivationFunctionType.Exp, scale=0.125,
                            )
                            for j in range(glen):
                                kt = kt0 + j
                                if "p" in phases and (kt % 3) != 0 and kt != NT - 1:
                                    continue
                                nc.tensor.matmul(
                                    o_e[0:DH + 1, :], V[:, kt, 2 * hp, :],
                                    p_e[:, j],
                                    start=(kt == 0), stop=(kt == NT - 1),
                                )
                                nc.tensor.matmul(
                                    o_o[0:DH + 1, :], V[:, kt, 2 * hp + 1, :],
                                    p_o[:, j],
                                    start=(kt == 0), stop=(kt == NT - 1),
                                )
                            kt0 += glen

                        # epilogue: drain O psum fast, then normalize -> HT
                        osb_eo = csb.tile([DH + 1, 2, NQ], F32, tag="osb")
                        nc.vector.tensor_copy(osb_eo[:, 0, :], o_e[0:DH + 1, :])
                        nc.vector.tensor_copy(osb_eo[:, 1, :], o_o[0:DH + 1, :])
                        rec = csb.tile([1, 2, NQ], F32, tag="rec")
                        nc.vector.tensor_copy(rec, osb_eo[DH:DH + 1, :, :])
                        nc.vector.reciprocal(rec, rec)
                        rbc_e = csb.tile([DH, NQ], F32, tag="rbce")
                        rbc_o = csb.tile([DH, NQ], F32, tag="rbco")
                        nc.gpsimd.partition_broadcast(rbc_e, rec[:, 0, :])
                        nc.gpsimd.partition_broadcast(rbc_o, rec[:, 1, :])
                        nc.vector.tensor_mul(
                            HT[0:DH, hp, qsl], osb_eo[0:DH, 0, :], rbc_e
                        )
                        nc.vector.tensor_mul(
                            HT[DH:P, hp, qsl], osb_eo[0:DH, 1, :], rbc_o
                        )

            # ---- Phase D: output projection ----
            if "D" in phases:
              with (
                tc.tile_pool(name=f"psD{rep}", bufs=4, space="PSUM") as psD,
                tc.tile_pool(name=f"osb{rep}", bufs=4) as osb,
              ):
                for kt in range(NT):
                    tsl = slice(kt * P, (kt + 1) * P)
                    po = psD.tile([P, 2, NQ], F32, tag="po")
                    ost = osb.tile([P, D], F32, tag="ost")
                    for nh in range(2):
                        for c in range(IC):
                            nc.tensor.matmul(
                                po[:, nh, 0:384],
                                HT[:, c, tsl],
                                wo_sb[:, c, nh * 384:(nh + 1) * 384],
                                start=(c == 0),
                                stop=(c == IC - 1),
                            )
                    nc.vector.tensor_copy(
                        ost.rearrange("p (n f) -> p n f", n=2), po[:, :, 0:384]
                    )
                    nc.sync.dma_start(out_d[tsl, :], ost)


def _get_nc():
    global _CACHED_NC
    if _CACHED_NC is None:
        _CACHED_NC = build_program()
    return _CACHED_NC


def kernel(x, w_q, w_k, w_v, w_o, b_o):
    from concourse.bass_utils import run_bass_kernel_spmd

    x = np.asarray(x, dtype=np.float32)
    w_q = np.asarray(w_q, dtype=np.float32)
    w_k = np.asarray(w_k, dtype=np.float32)
    w_v = np.asarray(w_v, dtype=np.float32)
    w_o = np.asarray(w_o, dtype=np.float32)
    b_o = np.asarray(b_o, dtype=np.float32)

    nc = _get_nc()
    in_maps = []
    for c in range(8):
        b = c // 2
        s = slice((c % 2) * INNER_L, (c % 2) * INNER_L + INNER_L)
        in_maps.append({
            "x": np.ascontiguousarray(x[b]),
            "w_q": np.ascontiguousarray(w_q[:, s]),
            "w_k": np.ascontiguousarray(w_k[:, s]),
            "w_v": np.ascontiguousarray(w_v[:, s]),
            "w_o": np.ascontiguousarray(w_o[s, :]),
        })
    res = run_bass_kernel_spmd(nc, in_maps, list(range(8)))
    out = np.zeros((B, N, D), np.float32)
    for c in range(8):
        out[c // 2] += res.results[c]["out"]
    out += b_o
    return out


if __name__ == "__main__":
    # quick self-check against a numpy reference
    rng = np.random.default_rng(0)
    ins = {
        "x": rng.standard_normal((B, N, D), dtype=np.float32),
        "w_q": (rng.standard_normal((D, D), dtype=np.float32) * 0.02),
        "w_k": (rng.standard_normal((D, D), dtype=np.float32) * 0.02),
        "w_v": (rng.standard_normal((D, D), dtype=np.float32) * 0.02),
        "w_o": (rng.standard_normal((D, D), dtype=np.float32) * 0.02),
        "b_o": np.zeros((D,), np.float32),
    }
    got = kernel(**ins)

    def ref(x, w_q, w_k, w_v, w_o, b_o):
        q = (x @ w_q).reshape(B, N, HEADS, DH).transpose(0, 2, 1, 3)
        k = (x @ w_k).reshape(B, N, HEADS, DH).transpose(0, 2, 1, 3)
        v = (x @ w_v).reshape(B, N, HEADS, DH).transpose(0, 2, 1, 3)
        s = np.einsum("bhnd,bhmd->bhnm", q, k) / 8.0
        s = s - s.max(axis=-1, keepdims=True)
        p = np.exp(s)
        p = p / p.sum(axis=-1, keepdims=True)
        h = np.einsum("bhnm,bhmd->bhnd", p, v)
        H = h.transpose(0, 2, 1, 3).reshape(B, N, HEADS * DH)
        return H @ w_o + b_o

    exp = ref(**ins)
    err = np.abs(got - exp)
    print(f"absmax err {err.max():.3e}  scale {np.abs(exp).max():.3e}  "
          f"rel {err.max() / np.abs(exp).max():.3e}")

